# revision 34
# baseline (speedup 1.0000x reference)
"""Trainium2 Bass kernel for BinaryConv2dLayer.

Reference op: W_b = sign(W) * (sum(W)/sum(sign(W))); y = relu(conv2d_SAME(x, W_b)).
x: [16, 256, 256, 64] NHWC fp32, W: [3, 3, 64, 64] HWIO fp32.

Strategy (data-parallel, 2 images per core on 8 cores):
- Host: binarize weights to exact +-1 (bf16-exact); the scalar `scale` is
  applied on-device in fp32 during the epilogue. x is cast to bf16 and laid
  out channel-major: partitions = (row-parity, 64 ch), free dim = flattened
  (row-pair, width-padded 258 cols), with zero halo pairs baked in so SAME
  padding and image boundaries need no special-casing on device.
- Device: gather-form conv as 6 accumulating K=128/M=128 N=512 matmuls per
  PSUM block. For each kernel-column shift dx: one "full" matmul packs the
  row-pair (2 input rows) against both output rows (dy in {-1,0} resp {0,1}),
  and one "boundary" matmul on a cross-shifted second slab (even rows loaded
  from +1 pair, odd rows from -1 pair) covers the remaining dy taps, with
  zero lhsT quadrants masking invalid row/output combinations. Epilogue: DVE
  fused scale+relu -> bf16, contiguous channel-major DMA store. Host
  transposes back to NHWC and upcasts to fp32.
Modeled (Tile cost model) exec time: ~178 us/core; PE busy ~165 us (93%),
DMA ~149 us. Verified vs the fp32 jax reference: rel L2 err ~2.4e-3 (bf16
input/output rounding).
"""

import numpy as np
import ml_dtypes

BF16 = ml_dtypes.bfloat16

H = 256
WD = 256
C = 64
PAIRS = H // 2            # 128 row pairs per image
COLW = WD + 2             # width + SAME padding cols
PAD = 4                   # extra zero slack at buffer ends
FL = 2 * PAD + COLW * (PAIRS + 4)     # per-image flat cols incl. 2 halo pairs/side
OUT0 = PAD + 2 * COLW     # flat col where pair 0 starts
OUTL = PAIRS * COLW       # per-image output cols (padded layout)
NIMG = 16
NCORES = 8
IPC = NIMG // NCORES      # images per core
P_SLAB = 32               # row pairs per SBUF slab
FIRST_PS = (8, 24, 32, 32, 32)   # slab schedule for the first image
NBLK = 512                # PSUM block width (one fp32 bank)
SLAB_BUFS = 3
PSUM_BUFS = 8
OUT_BUFS = 6
IN_DMA_SPLIT = 2          # column-chunks per slab DMA

_PROG = {}


def _build_program(scale):
    import concourse.mybir as mybir
    from concourse import bacc
    from concourse.tile import TileContext

    dt = mybir.dt
    nc = bacc.Bacc("TRN2")
    xflat = nc.dram_tensor("xflat", [128, IPC * FL], dt.bfloat16, kind="ExternalInput")
    wg = nc.dram_tensor("wg", [128, 3 * 128], dt.bfloat16, kind="ExternalInput")
    wb = nc.dram_tensor("wb", [128, 3 * 128], dt.bfloat16, kind="ExternalInput")
    y = nc.dram_tensor("y", [128, IPC * OUTL], dt.bfloat16, kind="ExternalOutput")

    SLAB_COLS = (max(max(FIRST_PS), P_SLAB) + 2) * COLW + 2 * PAD

    def chunked_dma(dst, dst_lo, dst_hi, src, src0, cols, n=None):
        n = n or IN_DMA_SPLIT
        step = (cols + n - 1) // n
        c = 0
        while c < cols:
            w = min(step, cols - c)
            nc.sync.dma_start(out=dst[dst_lo:dst_hi, c:c + w],
                              in_=src[dst_lo:dst_hi, src0 + c:src0 + c + w])
            c += w

    with TileContext(nc) as tc:
        with (
            tc.tile_pool(name="wpool", bufs=1) as wpool,
            tc.tile_pool(name="slab", bufs=SLAB_BUFS) as slabp,
            tc.tile_pool(name="psum", bufs=PSUM_BUFS, space="PSUM") as psump,
            tc.tile_pool(name="outp", bufs=OUT_BUFS) as outp,
        ):
            wg_t = wpool.tile([128, 3 * 128], dt.bfloat16)
            nc.sync.dma_start(out=wg_t[:], in_=wg[:])
            wb_t = wpool.tile([128, 3 * 128], dt.bfloat16)
            nc.sync.dma_start(out=wb_t[:], in_=wb[:])

            # smaller first slab so the PE pipeline fills sooner
            first_ps = list(FIRST_PS)
            assert sum(first_ps) == PAIRS
            rest_ps = [P_SLAB] * (PAIRS // P_SLAB)
            for img in range(IPC):
                r0 = 0
                for P in (first_ps if img == 0 else rest_ps):
                    # natural slab: both halves from the same window (pairs r0-1..r0+P)
                    scols = (P + 2) * COLW + 2 * PAD
                    a0 = img * FL + (r0 + 1) * COLW
                    slab = slabp.tile([128, SLAB_COLS], dt.bfloat16, tag="slab")
                    chunked_dma(slab, 0, 128, xflat, a0, scols)
                    # cross-shifted slab: even rows from +COLW, odd rows from -COLW.
                    # Used by the merged boundary matmuls (zero lhsT quadrants
                    # mask the half that doesn't apply).
                    slab2 = slabp.tile([128, SLAB_COLS], dt.bfloat16, tag="slab2")
                    chunked_dma(slab2, 0, 64, xflat, a0 + COLW, scols)
                    chunked_dma(slab2, 64, 128, xflat, a0 - COLW, scols)
                    t_start = OUT0 - COLW  # slab-local col of pair r0
                    for T in range(t_start, t_start + P * COLW, NBLK):
                        N = min(NBLK, t_start + P * COLW - T)
                        ps = psump.tile([128, NBLK], dt.float32)
                        # 3 full matmuls: K=128 (2 rows x 64ch), M=128 (2 out rows x 64 cout)
                        for dxi, dx in enumerate((-1, 0, 1)):
                            nc.tensor.matmul(
                                ps[:, :N],
                                wg_t[:, dxi * 128:(dxi + 1) * 128],
                                slab[:, T + dx:T + dx + N],
                                start=(dxi == 0),
                                stop=False,
                            )
                        # 3 merged boundary matmuls on the cross-shifted slab
                        for dxi, dx in enumerate((-1, 0, 1)):
                            nc.tensor.matmul(
                                ps[:, :N],
                                wb_t[:, dxi * 128:(dxi + 1) * 128],
                                slab2[:, T + dx:T + dx + N],
                                start=False,
                                stop=(dxi == 2),
                            )
                        ot = outp.tile([128, NBLK], dt.bfloat16)
                        nc.vector.tensor_scalar(
                            out=ot[:, :N],
                            in0=ps[:, :N],
                            scalar1=float(scale),
                            scalar2=0.0,
                            op0=mybir.AluOpType.mult,
                            op1=mybir.AluOpType.max,
                        )
                        dst0 = img * OUTL + r0 * COLW + (T - t_start)
                        nc.scalar.dma_start(out=y[:, dst0:dst0 + N], in_=ot[:, :N])
                    r0 += P
    nc.finalize()
    return nc


def _get_program(scale):
    key = float(scale)
    if key not in _PROG:
        _PROG[key] = _build_program(key)
    return _PROG[key]


def _host_prep_x(x):
    xb = np.ascontiguousarray(x).astype(BF16)
    xr = xb.reshape(NCORES, IPC, PAIRS, 2, WD, C)
    xflat = np.zeros((NCORES, 128, IPC * FL), dtype=BF16)
    for j in range(IPC):
        base = j * FL + OUT0
        view = xflat[:, :, base:base + PAIRS * COLW].reshape(NCORES, 128, PAIRS, COLW)
        for p in range(2):
            # [core, pair, w, c] -> [core, c, pair, w]
            view[:, 64 * p:64 * (p + 1), :, 1:257] = xr[:, j, :, p].transpose(0, 3, 1, 2)
    return xflat


def _host_prep_w(W):
    Wf = np.ascontiguousarray(W).astype(np.float32)
    sgn = np.sign(Wf)
    scale = np.float32(Wf.sum(dtype=np.float32) / sgn.sum(dtype=np.float32))
    sgn16 = sgn.astype(BF16)  # exact +-1
    wg = np.zeros((128, 3 * 128), dtype=BF16)
    wb = np.zeros((128, 3 * 128), dtype=BF16)
    for dxi in range(3):
        m = wg[:, dxi * 128:(dxi + 1) * 128]
        m[0:64, 0:64] = sgn16[1, dxi]      # even in -> even out (ky=1)
        m[64:128, 0:64] = sgn16[2, dxi]    # odd in -> even out (ky=2)
        m[0:64, 64:128] = sgn16[0, dxi]    # even in -> odd out (ky=0)
        m[64:128, 64:128] = sgn16[1, dxi]  # odd in -> odd out (ky=1)
        b = wb[:, dxi * 128:(dxi + 1) * 128]
        b[0:64, 64:128] = sgn16[2, dxi]    # even in of next pair -> odd out (ky=2)
        b[64:128, 0:64] = sgn16[0, dxi]    # odd in of prev pair -> even out (ky=0)
    return wg, wb, scale


def _unshard(results):
    out = np.empty((NIMG, H, WD, C), dtype=np.float32)
    for k in range(NCORES):
        yk = results[k]["y"]
        for j in range(IPC):
            o = yk[:, j * OUTL:(j + 1) * OUTL].reshape(2, 64, PAIRS, COLW)[:, :, :, 1:257]
            # [g, c, r, w] -> [r, g, w, c] -> [256, 256, 64]
            out[k * IPC + j] = (
                o.transpose(2, 0, 3, 1).reshape(H, WD, C).astype(np.float32)
            )
    return out


def kernel(x, W):
    from concourse.bass_utils import run_bass_kernel_spmd

    xflat = _host_prep_x(np.asarray(x))
    wg, wb, scale = _host_prep_w(np.asarray(W))
    nc = _get_program(scale)
    in_maps = [
        {"xflat": np.ascontiguousarray(xflat[k]), "wg": wg, "wb": wb}
        for k in range(NCORES)
    ]
    res = run_bass_kernel_spmd(nc, in_maps, core_ids=list(range(NCORES)))
    return _unshard(res.results)


# revision 52
# speedup vs baseline: 1.0164x; 1.0164x over previous
"""Trainium2 Bass kernel for BinaryConv2dLayer.

Reference op: W_b = sign(W) * (sum(W)/sum(sign(W))); y = relu(conv2d_SAME(x, W_b)).
x: [16, 256, 256, 64] NHWC fp32, W: [3, 3, 64, 64] HWIO fp32.

Strategy (data-parallel, 2 images per core on 8 cores):
- Host: binarize weights to exact +-1 (bf16-exact); the scalar `scale` is
  applied on-device in fp32 during the epilogue. x is cast to bf16 and laid
  out channel-major: partitions = (row-parity, 64 ch), free dim = flattened
  (row-pair, width-padded 258 cols), with zero halo pairs baked in so SAME
  padding and image boundaries need no special-casing on device.
- Device: gather-form conv as 6 accumulating K=128/M=128 N=512 matmuls per
  PSUM block. For each kernel-column shift dx: one "full" matmul packs the
  row-pair (2 input rows) against both output rows (dy in {-1,0} resp {0,1}),
  and one "boundary" matmul on a cross-shifted second slab (even rows loaded
  from +1 pair, odd rows from -1 pair) covers the remaining dy taps, with
  zero lhsT quadrants masking invalid row/output combinations. Epilogue: DVE
  fused scale+relu -> bf16, contiguous channel-major DMA store. Host
  transposes back to NHWC and upcasts to fp32.
Modeled (Tile cost model) exec time: ~175 us/core; PE busy ~165 us (95%),
DMA ~149 us. Residual stalls are fundamental: ~3 us startup DMA fill, ~2 us
transient DMA-bandwidth limit, ~4 us fixed Tile end-barrier. Verified vs the
fp32 jax reference: rel L2 err ~2.4e-3 (bf16 input/output rounding).
"""

import numpy as np
import ml_dtypes

BF16 = ml_dtypes.bfloat16

H = 256
WD = 256
C = 64
PAIRS = H // 2            # 128 row pairs per image
COLW = WD + 2             # width + SAME padding cols
PAD = 4                   # extra zero slack at buffer ends
FL = 2 * PAD + COLW * (PAIRS + 4)     # per-image flat cols incl. 2 halo pairs/side
OUT0 = PAD + 2 * COLW     # flat col where pair 0 starts
OUTL = PAIRS * COLW       # per-image output cols (padded layout)
NIMG = 16
NCORES = 8
IPC = NIMG // NCORES      # images per core
P_SLAB = 32               # row pairs per SBUF slab
FIRST_PS = (8, 24, 24, 24, 24, 24)   # slab schedule for the first image
REST_PS = (32, 32, 32, 32)           # slab schedule for later images
NBLK = 512                # PSUM block width (one fp32 bank)
SLAB_BUFS = 3
PSUM_BUFS = 8
OUT_BUFS = 6
IN_DMA_SPLIT = 2          # column-chunks per slab DMA

_PROG = {}


def _build_program(scale):
    import concourse.mybir as mybir
    from concourse import bacc
    from concourse.tile import TileContext

    dt = mybir.dt
    nc = bacc.Bacc("TRN2")
    xflat = nc.dram_tensor("xflat", [128, IPC * FL], dt.bfloat16, kind="ExternalInput")
    wg = nc.dram_tensor("wg", [128, 3 * 128], dt.bfloat16, kind="ExternalInput")
    wb = nc.dram_tensor("wb", [128, 3 * 128], dt.bfloat16, kind="ExternalInput")
    y = nc.dram_tensor("y", [128, IPC * OUTL], dt.bfloat16, kind="ExternalOutput")

    SLAB_COLS = (max(max(FIRST_PS), max(REST_PS)) + 2) * COLW + 2 * PAD

    with TileContext(nc) as tc:
        with (
            tc.tile_pool(name="wpool", bufs=1) as wpool,
            tc.tile_pool(name="slab", bufs=SLAB_BUFS) as slabp,
            tc.tile_pool(name="psum", bufs=PSUM_BUFS, space="PSUM") as psump,
            tc.tile_pool(name="outp", bufs=OUT_BUFS) as outp,
        ):
            wg_t = wpool.tile([128, 3 * 128], dt.bfloat16)
            nc.sync.dma_start(out=wg_t[:], in_=wg[:])
            wb_t = wpool.tile([128, 3 * 128], dt.bfloat16)
            nc.sync.dma_start(out=wb_t[:], in_=wb[:])

            # smaller first slab so the PE pipeline fills sooner
            first_ps = list(FIRST_PS)
            rest_ps = list(REST_PS)
            assert sum(first_ps) == PAIRS and sum(rest_ps) == PAIRS
            for img in range(IPC):
                r0 = 0
                for P in (first_ps if img == 0 else rest_ps):
                    # natural slab: both halves from the same window (pairs r0-1..r0+P)
                    scols = (P + 2) * COLW + 2 * PAD
                    a0 = img * FL + (r0 + 1) * COLW
                    slab = slabp.tile([128, SLAB_COLS], dt.bfloat16, tag="slab")
                    # cross-shifted slab: even rows from +COLW, odd rows from -COLW.
                    # Used by the merged boundary matmuls (zero lhsT quadrants
                    # mask the half that doesn't apply).
                    slab2 = slabp.tile([128, SLAB_COLS], dt.bfloat16, tag="slab2")
                    # round-robin the column chunks of all three loads so the
                    # leading chunks (which gate the first blocks) arrive first
                    step = (scols + IN_DMA_SPLIT - 1) // IN_DMA_SPLIT
                    for c in range(0, scols, step):
                        w = min(step, scols - c)
                        nc.sync.dma_start(out=slab[:, c:c + w],
                                          in_=xflat[:, a0 + c:a0 + c + w])
                        nc.sync.dma_start(out=slab2[0:64, c:c + w],
                                          in_=xflat[0:64, a0 + COLW + c:a0 + COLW + c + w])
                        nc.sync.dma_start(out=slab2[64:128, c:c + w],
                                          in_=xflat[64:128, a0 - COLW + c:a0 - COLW + c + w])
                    t_start = OUT0 - COLW  # slab-local col of pair r0
                    for T in range(t_start, t_start + P * COLW, NBLK):
                        N = min(NBLK, t_start + P * COLW - T)
                        ps = psump.tile([128, NBLK], dt.float32)
                        # 3 full matmuls: K=128 (2 rows x 64ch), M=128 (2 out rows x 64 cout)
                        for dxi, dx in enumerate((-1, 0, 1)):
                            nc.tensor.matmul(
                                ps[:, :N],
                                wg_t[:, dxi * 128:(dxi + 1) * 128],
                                slab[:, T + dx:T + dx + N],
                                start=(dxi == 0),
                                stop=False,
                            )
                        # 3 merged boundary matmuls on the cross-shifted slab
                        for dxi, dx in enumerate((-1, 0, 1)):
                            nc.tensor.matmul(
                                ps[:, :N],
                                wb_t[:, dxi * 128:(dxi + 1) * 128],
                                slab2[:, T + dx:T + dx + N],
                                start=False,
                                stop=(dxi == 2),
                            )
                        ot = outp.tile([128, NBLK], dt.bfloat16)
                        nc.vector.tensor_scalar(
                            out=ot[:, :N],
                            in0=ps[:, :N],
                            scalar1=float(scale),
                            scalar2=0.0,
                            op0=mybir.AluOpType.mult,
                            op1=mybir.AluOpType.max,
                        )
                        dst0 = img * OUTL + r0 * COLW + (T - t_start)
                        nc.scalar.dma_start(out=y[:, dst0:dst0 + N], in_=ot[:, :N])
                    r0 += P
    nc.finalize()
    return nc


def _get_program(scale):
    key = float(scale)
    if key not in _PROG:
        _PROG[key] = _build_program(key)
    return _PROG[key]


def _host_prep_x(x):
    xb = np.ascontiguousarray(x).astype(BF16)
    xr = xb.reshape(NCORES, IPC, PAIRS, 2, WD, C)
    xflat = np.zeros((NCORES, 128, IPC * FL), dtype=BF16)
    for j in range(IPC):
        base = j * FL + OUT0
        view = xflat[:, :, base:base + PAIRS * COLW].reshape(NCORES, 128, PAIRS, COLW)
        for p in range(2):
            # [core, pair, w, c] -> [core, c, pair, w]
            view[:, 64 * p:64 * (p + 1), :, 1:257] = xr[:, j, :, p].transpose(0, 3, 1, 2)
    return xflat


def _host_prep_w(W):
    Wf = np.ascontiguousarray(W).astype(np.float32)
    sgn = np.sign(Wf)
    scale = np.float32(Wf.sum(dtype=np.float32) / sgn.sum(dtype=np.float32))
    sgn16 = sgn.astype(BF16)  # exact +-1
    wg = np.zeros((128, 3 * 128), dtype=BF16)
    wb = np.zeros((128, 3 * 128), dtype=BF16)
    for dxi in range(3):
        m = wg[:, dxi * 128:(dxi + 1) * 128]
        m[0:64, 0:64] = sgn16[1, dxi]      # even in -> even out (ky=1)
        m[64:128, 0:64] = sgn16[2, dxi]    # odd in -> even out (ky=2)
        m[0:64, 64:128] = sgn16[0, dxi]    # even in -> odd out (ky=0)
        m[64:128, 64:128] = sgn16[1, dxi]  # odd in -> odd out (ky=1)
        b = wb[:, dxi * 128:(dxi + 1) * 128]
        b[0:64, 64:128] = sgn16[2, dxi]    # even in of next pair -> odd out (ky=2)
        b[64:128, 0:64] = sgn16[0, dxi]    # odd in of prev pair -> even out (ky=0)
    return wg, wb, scale


def _unshard(results):
    out = np.empty((NIMG, H, WD, C), dtype=np.float32)
    for k in range(NCORES):
        yk = results[k]["y"]
        for j in range(IPC):
            o = yk[:, j * OUTL:(j + 1) * OUTL].reshape(2, 64, PAIRS, COLW)[:, :, :, 1:257]
            # [g, c, r, w] -> [r, g, w, c] -> [256, 256, 64]
            out[k * IPC + j] = (
                o.transpose(2, 0, 3, 1).reshape(H, WD, C).astype(np.float32)
            )
    return out


def kernel(x, W):
    from concourse.bass_utils import run_bass_kernel_spmd

    xflat = _host_prep_x(np.asarray(x))
    wg, wb, scale = _host_prep_w(np.asarray(W))
    nc = _get_program(scale)
    in_maps = [
        {"xflat": np.ascontiguousarray(xflat[k]), "wg": wg, "wb": wb}
        for k in range(NCORES)
    ]
    res = run_bass_kernel_spmd(nc, in_maps, core_ids=list(range(NCORES)))
    return _unshard(res.results)
